# revision 33
# baseline (speedup 1.0000x reference)
"""DispersionLoss kernel for Trainium2 (8 NeuronCores, Bass/Tile).

Reference computation (N=16384, F=64, K=32, C=128):
    bin_mass[f,k]  = sum_n m[n,f,k] + EPS
    SWY[f,k,c]     = sum_n m[n,f,k] * y[n,c]
    cent[f,k,c]    = SWY / bin_mass
    loss_dispersion= sum_fk (A/bin_mass - c_sq)   [EPS*c_sq/bin_mass ~1e-11, dropped]
        where A[f,k] = sum_n m[n,f,k]*|y_n|^2
    loss_entropy   = sum_fk p*log(p+EPS), p = bin_mass/N  (host, from shipped bin_mass)
    loss_repulsion = sum_f sum_k exp(-|cent[f,k]-cent[f,k+1]|^2)
    loss_inter     = sum_f (sum_{kj} exp(-pairwise) - K) / 2 / F

Sharding: over F (8 features per core) -> every loss term decomposes per-f,
no cross-core collectives; host sums 8 partial vectors.

v4 design (vs the 41.5us single-queue baseline):
  - inputs quantized to fp8 e4m3 on host; ysq shipped as fp8 hi+lo pair.
  - g repacked into TWO bin-half slabs (h=0: bins 0..127 = features 0..3,
    h=1: bins 128..255).  All of slab0 streams before slab1, so the h=0
    half finishes its PSUM accumulation ~2/3 through the DMA span; its
    centroid/ccT/sqf/dd prep runs on vector + a DMA-transpose (scalar
    queue, behind y) with ZERO tensor-engine involvement, fully
    overlapped with slab1's DMA + matmuls.
  - input DMA split across both HWDGE queues: y (5 chunks) on the scalar
    queue, g (4 slab0 blocks + 8 slab1 half-blocks) on the sync queue
    -> ~380 GB/s aggregate; finer slab1 granularity shortens the final
    data-gated matmul burst.
  - no Ln on device: bin_mass (128,2) is DMA'd out on the sync queue and
    the tiny (F,K) entropy term is computed on host in f64.  Scalar
    engine runs ONLY the four Exp ACTs (table loaded once, no reloads);
    cent16/ccT/qneg/nshift all run on the vector engine via fused
    tensor_scalar ops.
  - per-half pairwise (128x128, within-half; halves never share a feature)
    with the -B cross-feature block bias so exp() zeroes them and the Exp
    ACT's accum_out yields the inter-loss block sums for free.
  - HAM: wide (512-col) junk matmuls absorb the ~3.4us cold ramp and fill
    slab0's DMA pacing gaps so real DR matmuls run at 2.4 GHz; the merged
    endgame keeps the PE active enough to stay unlocked.
  - endgame: the two half-tails are interleaved per-engine in expected
    ready-time order, so h0's remaining PE work (psq/pairwise/rank-1) and
    scalar Exps slot into h1's dependency stalls.
"""

import numpy as np

N = 16384
F = 64
K = 32
C = 128
NCORES = 8
F_PER_CORE = F // NCORES          # 8
FK = F_PER_CORE * K               # 256 bins per core
NPAIR = N // 256                  # 64 subtile pairs (DoubleRow: 256 rows/mm)
W = 132                           # moving cols: [y(128) | 1 | ysq_h | ysq_l | pad]
PPB = 16                          # pairs per g block (DRAM layout)
NBLK = NPAIR // PPB               # 4 blocks per slab
CSC = 16.0                        # centered-centroid scale (keeps fp16 normal)
BBIAS = 3840.0                    # cross-feature psE bias: exp arg -= 30
NWARM = 8                         # upfront wide junk MMs (~3.4us cold ramp)
JGAP = 4                          # wide junk MMs per slab0 inter-block gap

LAMBDA_ENTROPY = 0.1
LAMBDA_REPULSION = 0.5
LAMBDA_INTER = 0.3
EPS = 1e-8

_NC_CACHE = {}


def _f8dtype():
    import ml_dtypes
    return ml_dtypes.float8_e4m3


def _pack_g(gc: np.ndarray) -> np.ndarray:
    """(N, FK) fp8 -> (8*128, PPB*2*128): slab h (4 blocks), block row p holds,
    for the 16 pairs u of the block, [i=0 | i=1] x 128 half-bins where the
    n-row is 256*u + 128*i + p."""
    x = gc.reshape(NPAIR, 2, 128, 2, 128)           # u, i, p, h, fk
    x = x.reshape(NBLK, PPB, 2, 128, 2, 128)        # blk, ul, i, p, h, fk
    x = x.transpose(4, 0, 3, 1, 2, 5)               # h, blk, p, ul, i, fk
    return np.ascontiguousarray(x.reshape(2 * NBLK * 128, PPB * 2 * 128))


def _pack_y(yslab: np.ndarray) -> np.ndarray:
    """(N, W) fp8 -> (128, NPAIR*2*W): partition p holds pair-major slabs."""
    return np.ascontiguousarray(
        yslab.reshape(NPAIR, 2, 128, W).transpose(2, 0, 1, 3).reshape(128, NPAIR * 2 * W)
    )


def _finalize(parts: np.ndarray, masses: np.ndarray):
    """parts: (ncores, 128, 8): [wv0, wv1, eall0, eall1, rep0, rep1, rx0, rx1].
    masses: (ncores, 128, 2) = bin_mass (+EPS) per half."""
    r = parts.astype(np.float64).sum(axis=(0, 1))
    disp = r[0] + r[1]
    p = masses.astype(np.float64).reshape(-1) / N
    ent = float(np.sum(p * np.log(p + EPS)))
    rep = (r[4] + r[5]) - (r[6] + r[7])
    inter = (r[2] + r[3] - F * K) / (2.0 * F)
    tot = disp + LAMBDA_ENTROPY * ent + LAMBDA_REPULSION * rep + LAMBDA_INTER * inter
    return tuple(np.float32(v) for v in (tot, disp, ent, rep, inter))


def _build_nc():
    import concourse.bacc as bacc
    import concourse.tile as tile
    from concourse import mybir

    f32 = mybir.dt.float32
    f16 = mybir.dt.float16
    f8 = mybir.dt.float8e4
    DR = mybir.MatmulPerfMode.DoubleRow
    AF = mybir.ActivationFunctionType
    OP = mybir.AluOpType

    nc = bacc.Bacc("TRN2", target_bir_lowering=False, debug=False,
                   enable_asserts=False, enable_partition_id=False)
    g_dram = nc.dram_tensor("g", (2 * NBLK * 128, PPB * 2 * 128), f8,
                            kind="ExternalInput").ap()
    y_dram = nc.dram_tensor("y", (128, NPAIR * 2 * W), f8, kind="ExternalInput").ap()
    out_dram = nc.dram_tensor("out", (128, 8), f32, kind="ExternalOutput").ap()
    mass_dram = nc.dram_tensor("mass", (128, 2), f32, kind="ExternalOutput").ap()

    with tile.TileContext(nc) as tc:
        with (
            tc.tile_pool(name="singles", bufs=1) as singles,
            tc.tile_pool(name="gpool", bufs=8) as gpool,
            tc.tile_pool(name="scr", bufs=2) as scr,
            tc.tile_pool(name="ph2", bufs=1) as ph2,
            tc.tile_pool(name="psacc", bufs=1, space="PSUM") as psacc,
            tc.tile_pool(name="pstmp", bufs=1, space="PSUM") as pstmp,
        ):
            yres = singles.tile([128, NPAIR * 2 * W], f8, name="yres")

            # ---- input DMA: y on the scalar HWDGE queue, g on the sync
            # HWDGE queue (~380 GB/s aggregate).  16 input transfers; the
            # 8 HWDGE sem lanes recycle input->input only (every lane
            # predecessor is a pure input transfer that completes in FIFO
            # order), so no issue ever waits on late compute.  Graduated
            # first chunks (4/12/16 pairs) let the first matmuls start
            # ~2us earlier; slab1's last two blocks are split in half so
            # the final data-gated burst is only 8 pairs.
            gtiles = [gpool.tile([128, PPB * 2 * 128], f8, name="g")
                      for _ in range(2 * NBLK)]

            def dma_g(blk, plo, phi):
                nc.sync.dma_start(
                    out=gtiles[blk][:, plo * 256:phi * 256],
                    in_=g_dram[blk * 128:(blk + 1) * 128, plo * 256:phi * 256])

            def dma_y(plo, phi):
                nc.scalar.dma_start(out=yres[:, plo * 2 * W:phi * 2 * W],
                                    in_=y_dram[:, plo * 2 * W:phi * 2 * W])

            def dma_g_sc(blk, plo, phi):
                nc.scalar.dma_start(
                    out=gtiles[blk][:, plo * 256:phi * 256],
                    in_=g_dram[blk * 128:(blk + 1) * 128, plo * 256:phi * 256])

            dma_g(0, 0, 2)
            dma_y(0, 2)
            dma_g(0, 2, 16)
            dma_y(2, 16)
            dma_g(1, 0, 16)
            dma_y(16, 32)
            dma_g(2, 0, 16)
            dma_y(32, 48)
            dma_g(3, 0, 16)
            dma_y(48, 56)
            dma_y(56, 64)
            # slab1 8-pair sub-blocks alternate sync/scalar so both queues
            # stay busy to the end and arrivals are monotone in MM order
            for blk in range(NBLK, 2 * NBLK):
                dma_g(blk, 0, 8)
                dma_g_sc(blk, 8, 16)

            # ---- PE junk source (wide: 512 moving cols ~ 213ns warm) ----
            wsrc = singles.tile([128, 512], f16)
            nc.vector.memset(wsrc, 0.0)
            wps = pstmp.tile([128, 512], f32, tag="psT0", name="warmps")

            def emit_junk(n):
                for _ in range(n):
                    nc.tensor.matmul(wps, wsrc[:, 0:128], wsrc,
                                     start=True, stop=True)


            # ---- constants ----
            ones128 = singles.tile([128, 1], f32)
            nc.gpsimd.memset(ones128, 1.0)
            eps128 = singles.tile([128, 1], f32)
            nc.gpsimd.memset(eps128, EPS)
            ones16c = singles.tile([128, 1], f16)
            nc.gpsimd.memset(ones16c, 1.0)
            ones_row = singles.tile([1, 128], f16)
            nc.gpsimd.memset(ones_row, 1.0)
            mhalf16 = singles.tile([128, 1], f16)
            nc.gpsimd.memset(mhalf16, -0.5)
            qneg_sb = singles.tile([1, 2 * 128], f16)
            id16 = singles.tile([128, 128], f16)
            nc.gpsimd.memset(id16, 0.0)
            nc.gpsimd.affine_select(
                out=id16, in_=id16,
                compare_op=OP.not_equal,
                fill=1.0, base=0, pattern=[[-1, 128]], channel_multiplier=1,
            )
            # CSC-scaled identity: transpose matmuls then yield CSC*cent^T
            # directly, so centering is a single fused subtract.
            id16csc = singles.tile([128, 128], f16)
            nc.gpsimd.memset(id16csc, 0.0)
            nc.gpsimd.affine_select(
                out=id16csc, in_=id16csc,
                compare_op=OP.not_equal,
                fill=CSC, base=0, pattern=[[-1, 128]], channel_multiplier=1,
            )
            # per-half feature indicator [4, 128] and cross-feature bias
            # -B*(1-ind); identical for both halves (local structure).
            ind16 = singles.tile([4, 128], f16)
            nc.gpsimd.memset(ind16, 0.0)
            i3 = ind16.rearrange("p (blk c) -> p blk c", c=32)
            nc.gpsimd.affine_select(
                out=i3, in_=i3, compare_op=OP.not_equal,
                fill=1.0, base=0, pattern=[[1, 4], [0, 32]],
                channel_multiplier=-1,
            )
            indB = singles.tile([4, 128], f16)
            nc.gpsimd.memset(indB, -BBIAS)
            b3 = indB.rearrange("p (blk c) -> p blk c", c=32)
            nc.gpsimd.affine_select(
                out=b3, in_=b3, compare_op=OP.not_equal,
                fill=0.0, base=0, pattern=[[1, 4], [0, 32]],
                channel_multiplier=-1,
            )
            # st cols: [wv0, wv1, eall0, eall1, rep0, rep1, rx0, rx1]
            st = ph2.tile([128, 8], f32)
            nc.gpsimd.memset(st, 0.0)

            # ---- preload the Exp table once; nothing else uses a table ----
            warm = ph2.tile([1, 2], f32)
            nc.scalar.activation(out=warm[0:1, 0:1], in_=ones128[0:1, 0:1], func=AF.Exp)

            # ---- phase 1: slab-ordered DoubleRow accumulation ----
            # ps[h][:, 0:128]=SWY_h, [:,128]=mass_raw, [:,129:131]=A_hi/lo
            ps = [psacc.tile([128, W], f32, name=f"acc{h}") for h in range(2)]

            # ---- tail tiles ----
            mass = ph2.tile([128, 2], f32)
            inv = ph2.tile([128, 2], f32)
            a_ = ph2.tile([128, 2], f32)
            csq = ph2.tile([128, 2], f32)
            t1 = ph2.tile([128, 2], f32)
            cent16 = ph2.tile([128, 2 * 128], f16)
            ccT = ph2.tile([128, 2 * 128], f16)
            sqc = scr.tile([128, 2 * 128], f16, tag="sqc")
            sqf = [scr.tile([128, 128], f16, tag=f"sqf{h}", name=f"sqf{h}")
                   for h in range(2)]
            dd = [scr.tile([128, 127], f16, tag=f"dd{h}", name=f"dd{h}")
                  for h in range(2)]
            en = [ph2.tile([1, 127], f32, name=f"en{h}") for h in range(2)]

            def v_mass_inv(h):
                nc.vector.tensor_scalar_add(
                    mass[:, h:h + 1], in0=ps[h][:, 128:129], scalar1=eps128)
                nc.vector.reciprocal(inv[:, h:h + 1], mass[:, h:h + 1])

            def v_cent(h):
                hs = slice(h * 128, (h + 1) * 128)
                with nc.allow_low_precision(reason="cent fp16 for exp"):
                    nc.vector.tensor_scalar_mul(
                        cent16[:, hs], in0=ps[h][:, 0:128], scalar1=inv[:, h:h + 1])

            def v_cc_from(h, src):
                # ccT = src - src[:,0:1]  (src = CSC*cent^T via id16csc)
                hs = slice(h * 128, (h + 1) * 128)
                with nc.allow_low_precision(reason="cc fp16 for exp"):
                    nc.vector.tensor_scalar_sub(ccT[:, hs], in0=src,
                                                scalar1=src[:, 0:1])

            def v_sqf_dd(h):
                hs = slice(h * 128, (h + 1) * 128)
                with nc.allow_low_precision(reason="scaled cc^2 fits fp16"):
                    nc.vector.tensor_mul(sqf[h], ccT[:, hs], ccT[:, hs])
                    nc.vector.tensor_sub(dd[h], ccT[:, h * 128:h * 128 + 127],
                                         ccT[:, h * 128 + 1:(h + 1) * 128])
                    nc.vector.tensor_mul(dd[h], dd[h], dd[h])

            pe_q = [None, None]
            pe_e = [None, None]
            pe_nd = [None, None]

            def p_qe(h):
                hs = slice(h * 128, (h + 1) * 128)
                pe_q[h] = pstmp.tile([1, 128], f32, tag="psq", name=f"psq{h}")
                nc.tensor.matmul(pe_q[h], mhalf16, sqf[h], start=True, stop=True)
                pe_e[h] = pstmp.tile([128, 128], f32, tag=f"psE{h}", name=f"psE{h}")
                nc.tensor.matmul(pe_e[h], ccT[:, hs], ccT[:, hs],
                                 start=True, stop=False)
                nc.tensor.matmul(pe_e[h], ind16, indB, start=False, stop=False)
                nd_t = pstmp.tile([1, 128], f32, tag="psq", name=f"psnd{h}")
                pe_nd[h] = nd_t[0:1, 0:127]
                nc.tensor.matmul(pe_nd[h], ones16c, dd[h], start=True, stop=True)

            def v_qneg(h):
                with nc.allow_low_precision(reason="q fp16 rank-1 operand"):
                    nc.vector.tensor_copy(qneg_sb[0:1, h * 128:(h + 1) * 128],
                                          pe_q[h])

            def s_qneg(h):
                with nc.allow_low_precision(reason="q fp16 rank-1 operand"):
                    nc.scalar.activation(
                        out=qneg_sb[0:1, h * 128:(h + 1) * 128],
                        in_=pe_q[h], func=AF.Copy)

            def p_rank1(h):
                qn = qneg_sb[0:1, h * 128:(h + 1) * 128]
                nc.tensor.matmul(pe_e[h], ones_row, qn, start=False, stop=False)
                nc.tensor.matmul(pe_e[h], qn, ones_row, start=False, stop=True)

            def s_exps(h):
                nc.scalar.activation(out=en[h], in_=pe_nd[h], func=AF.Exp,
                                     scale=-1.0 / (CSC * CSC),
                                     accum_out=st[0:1, 4 + h:5 + h])
                e_full = scr.tile([128, 128], f16, tag=f"ef{h}", name=f"ef{h}")
                with nc.allow_low_precision(reason="E<=1 fp16; accum f32"):
                    nc.scalar.activation(out=e_full, in_=pe_e[h], func=AF.Exp,
                                         scale=2.0 / (CSC * CSC),
                                         accum_out=st[:, 2 + h:3 + h])

            def v_stats(h):
                hs = slice(h * 128, (h + 1) * 128)
                xview = en[h][0:1, 31:31 + 96].rearrange("p (m c) -> p m c", c=32)
                nc.vector.reduce_sum(st[0:1, 6 + h:7 + h], xview[:, :, 0:1],
                                     axis=mybir.AxisListType.XY)
                nc.vector.reduce_sum(
                    a_[:, h:h + 1],
                    ps[h][:, 129:131].rearrange("p (one c) -> p one c", one=1),
                    axis=mybir.AxisListType.X)
                with nc.allow_low_precision(reason="csq via fp16 cent"):
                    nc.vector.tensor_mul(sqc[:, hs], cent16[:, hs], cent16[:, hs])
                nc.vector.reduce_sum(
                    csq[:, h:h + 1],
                    sqc[:, hs].rearrange("p (one c) -> p one c", one=1),
                    axis=mybir.AxisListType.X)
                nc.vector.tensor_mul(t1[:, h:h + 1], a_[:, h:h + 1],
                                     inv[:, h:h + 1])
                nc.vector.tensor_sub(st[:, h:h + 1], t1[:, h:h + 1],
                                     csq[:, h:h + 1])

            def emit_mm(u, h, start=False, stop=False):
                blk, ul = divmod(u, PPB)
                g = gtiles[h * NBLK + blk]
                gv = g[:, ul * 256:(ul + 1) * 256].rearrange(
                    "p (i fk) -> p i fk", i=2)
                yv = yres[:, u * 2 * W:(u + 1) * 2 * W].rearrange(
                    "p (i w) -> p i w", i=2)
                nc.tensor.matmul(
                    ps[h], gv, yv, start=start, stop=stop, perf_mode=DR,
                )

            def mm_range(lo, hi, h, start=False, stop=False):
                for u in range(lo, hi):
                    emit_mm(u, h, start=(start and u == lo),
                            stop=(stop and u == hi - 1))

            # slab1-era junk uses the psE1 bank (free until pe1 ~26us) so
            # it can never stall behind h0-tail reads of psT0.
            wps2 = pstmp.tile([128, 512], f32, tag="psE1", name="warmps2")

            def emit_junk2(n):
                for _ in range(n):
                    nc.tensor.matmul(wps2, wsrc[:, 0:128], wsrc,
                                     start=True, stop=True)

            # ---- slab0 (junk fills DMA pacing gaps) ----
            # no gap junk: after the upfront ramp unlocks the clock, the
            # natural ~55-70% duty through slab0 is active enough to avoid
            # the idle re-throttle but below the sustained-load duty cap
            # (continuous 100% activity triggers K=4 after one ~6.8us epoch)
            mm_range(0, 2, 0, start=True)       # b0a
            mm_range(2, 16, 0)                  # b0b
            mm_range(16, 32, 0)                 # b1
            mm_range(32, 48, 0)                 # b2
            mm_range(48, 64, 0, stop=True)      # b3 (y-gated trickle)

            # ---- h0 tail prefix: vector chain + one PE transpose ----
            v_mass_inv(0)
            v_cent(0)
            ps_t0 = pstmp.tile([128, 128], f32, tag="psT0", name="psT0")
            nc.tensor.matmul(ps_t0, cent16[:, 0:128], id16csc, start=True, stop=True)
            v_cc_from(0, ps_t0)
            v_sqf_dd(0)

            # ---- slab1 stream with h0's remaining tail ops slotted in ----
            mm_range(0, 32, 1, start=True)
            p_qe(0)                             # PE: psq0, pe0 dots+bias, nd0
            v_qneg(0)                           # vector (after psq0)
            mm_range(32, 48, 1)
            p_rank1(0)                          # PE (after qneg0): pe0 stop
            s_exps(0)                           # scalar: en0, ef0 (+accums)
            mm_range(48, 56, 1)
            v_stats(0)                          # vector (off-critical)
            mm_range(56, 64, 1, stop=True)

            # ---- endgame: h1 half-tail only ----
            v_mass_inv(1)                       # vector
            nc.sync.dma_start(out=mass_dram, in_=mass)
            v_cent(1)                           # vector
            ps_t1 = pstmp.tile([128, 128], f32, tag="psT1", name="psT1")
            nc.tensor.matmul(ps_t1, cent16[:, 128:256], id16csc, start=True, stop=True)
            v_cc_from(1, ps_t1)                 # vector: nshift1, ccT1
            v_sqf_dd(1)                         # vector
            p_qe(1)                             # PE: psq1, pe1 dots+bias, nd1
            s_qneg(1)                           # scalar (vector busy with dd1)
            p_rank1(1)                          # PE: pe1 stop
            s_exps(1)                           # scalar: en1, ef1 (+accums)
            v_stats(1)                          # vector

            nc.sync.dma_start(out=out_dram, in_=st)

    nc.compile()
    return nc


def get_nc():
    if "v29" not in _NC_CACHE:
        _NC_CACHE["v29"] = _build_nc()
    return _NC_CACHE["v29"]


def kernel(membership: np.ndarray, teacher_preds: np.ndarray, _trace: bool = False):
    from concourse.bass_utils import run_bass_kernel_spmd

    f8 = _f8dtype()
    m = np.asarray(membership, dtype=np.float32).reshape(N, F * K)
    y32 = np.asarray(teacher_preds, dtype=np.float32)
    ysq = np.einsum("nc,nc->n", y32, y32, dtype=np.float64).astype(np.float32)
    ysq_h = ysq.astype(f8)
    ysq_l = (ysq - ysq_h.astype(np.float32)).astype(f8)
    yslab = np.zeros((N, W), dtype=f8)
    yslab[:, 0:C] = y32.astype(f8)
    yslab[:, C] = np.float32(1.0)
    yslab[:, C + 1] = ysq_h
    yslab[:, C + 2] = ysq_l
    ypacked = _pack_y(yslab)

    m8 = m.astype(f8)
    nc = get_nc()
    in_maps = []
    for i in range(NCORES):
        in_maps.append({
            "g": _pack_g(m8[:, i * FK:(i + 1) * FK]),
            "y": ypacked,
        })
    res = run_bass_kernel_spmd(
        nc, in_maps, core_ids=list(range(NCORES)), trace=_trace,
    )
    parts = np.stack(
        [np.asarray(res.results[i]["out"], dtype=np.float64) for i in range(NCORES)]
    )
    masses = np.stack(
        [np.asarray(res.results[i]["mass"], dtype=np.float64) for i in range(NCORES)]
    )
    out = _finalize(parts, masses)
    if _trace:
        return out, res
    return out


if __name__ == "__main__":
    rng = np.random.default_rng(0)
    mem = rng.random((N, F, K), dtype=np.float32)
    tp = rng.random((N, C), dtype=np.float32)
    print(kernel(mem, tp))


# revision 34
# speedup vs baseline: 1.0597x; 1.0597x over previous
"""DispersionLoss kernel for Trainium2 (8 NeuronCores, Bass/Tile).

Reference computation (N=16384, F=64, K=32, C=128):
    bin_mass[f,k]  = sum_n m[n,f,k] + EPS
    SWY[f,k,c]     = sum_n m[n,f,k] * y[n,c]
    cent[f,k,c]    = SWY / bin_mass
    loss_dispersion= sum_fk (A/bin_mass - c_sq)   [EPS*c_sq/bin_mass ~1e-11, dropped]
        where A[f,k] = sum_n m[n,f,k]*|y_n|^2
    loss_entropy   = sum_fk p*log(p+EPS), p = bin_mass/N  (host, from shipped bin_mass)
    loss_repulsion = sum_f sum_k exp(-|cent[f,k]-cent[f,k+1]|^2)
    loss_inter     = sum_f (sum_{kj} exp(-pairwise) - K) / 2 / F

Sharding: over F (8 features per core) -> every loss term decomposes per-f,
no cross-core collectives; host sums 8 partial vectors.

v4 design (vs the 41.5us single-queue baseline):
  - inputs quantized to fp8 e4m3 on host; ysq shipped as fp8 hi+lo pair.
  - g repacked into TWO bin-half slabs (h=0: bins 0..127 = features 0..3,
    h=1: bins 128..255).  All of slab0 streams before slab1, so the h=0
    half finishes its PSUM accumulation ~2/3 through the DMA span; its
    centroid/ccT/sqf/dd prep runs on vector + a DMA-transpose (scalar
    queue, behind y) with ZERO tensor-engine involvement, fully
    overlapped with slab1's DMA + matmuls.
  - input DMA split across both HWDGE queues: y (5 chunks) on the scalar
    queue, g (4 slab0 blocks + 8 slab1 half-blocks) on the sync queue
    -> ~380 GB/s aggregate; finer slab1 granularity shortens the final
    data-gated matmul burst.
  - no Ln on device: bin_mass (128,2) is DMA'd out on the sync queue and
    the tiny (F,K) entropy term is computed on host in f64.  Scalar
    engine runs ONLY the four Exp ACTs (table loaded once, no reloads);
    cent16/ccT/qneg/nshift all run on the vector engine via fused
    tensor_scalar ops.
  - per-half pairwise (128x128, within-half; halves never share a feature)
    with the -B cross-feature block bias so exp() zeroes them and the Exp
    ACT's accum_out yields the inter-loss block sums for free.
  - HAM: wide (512-col) junk matmuls absorb the ~3.4us cold ramp and fill
    slab0's DMA pacing gaps so real DR matmuls run at 2.4 GHz; the merged
    endgame keeps the PE active enough to stay unlocked.
  - endgame: the two half-tails are interleaved per-engine in expected
    ready-time order, so h0's remaining PE work (psq/pairwise/rank-1) and
    scalar Exps slot into h1's dependency stalls.
"""

import numpy as np

N = 16384
F = 64
K = 32
C = 128
NCORES = 8
F_PER_CORE = F // NCORES          # 8
FK = F_PER_CORE * K               # 256 bins per core
NPAIR = N // 256                  # 64 subtile pairs (DoubleRow: 256 rows/mm)
W = 132                           # moving cols: [y(128) | 1 | ysq_h | ysq_l | pad]
PPB = 16                          # pairs per g block (DRAM layout)
NBLK = NPAIR // PPB               # 4 blocks per slab
CSC = 16.0                        # centered-centroid scale (keeps fp16 normal)
BBIAS = 3840.0                    # cross-feature psE bias: exp arg -= 30
NWARM = 8                         # upfront wide junk MMs (~3.4us cold ramp)
JGAP = 4                          # wide junk MMs per slab0 inter-block gap

LAMBDA_ENTROPY = 0.1
LAMBDA_REPULSION = 0.5
LAMBDA_INTER = 0.3
EPS = 1e-8

_NC_CACHE = {}


def _f8dtype():
    import ml_dtypes
    return ml_dtypes.float8_e4m3


def _pack_g(gc: np.ndarray) -> np.ndarray:
    """(N, FK) fp8 -> (8*128, PPB*2*128): slab h (4 blocks), block row p holds,
    for the 16 pairs u of the block, [i=0 | i=1] x 128 half-bins where the
    n-row is 256*u + 128*i + p."""
    x = gc.reshape(NPAIR, 2, 128, 2, 128)           # u, i, p, h, fk
    x = x.reshape(NBLK, PPB, 2, 128, 2, 128)        # blk, ul, i, p, h, fk
    x = x.transpose(4, 0, 3, 1, 2, 5)               # h, blk, p, ul, i, fk
    return np.ascontiguousarray(x.reshape(2 * NBLK * 128, PPB * 2 * 128))


def _pack_y(yslab: np.ndarray) -> np.ndarray:
    """(N, W) fp8 -> (128, NPAIR*2*W): partition p holds pair-major slabs."""
    return np.ascontiguousarray(
        yslab.reshape(NPAIR, 2, 128, W).transpose(2, 0, 1, 3).reshape(128, NPAIR * 2 * W)
    )


def _finalize(parts: np.ndarray, masses: np.ndarray):
    """parts: (ncores, 128, 8): [wv0, wv1, eall0, eall1, rep0, rep1, rx0, rx1].
    masses: (ncores, 128, 2) = bin_mass (+EPS) per half."""
    r = parts.astype(np.float64).sum(axis=(0, 1))
    disp = r[0] + r[1]
    p = masses.astype(np.float64).reshape(-1) / N
    ent = float(np.sum(p * np.log(p + EPS)))
    rep = (r[4] + r[5]) - (r[6] + r[7])
    inter = (r[2] + r[3] - F * K) / (2.0 * F)
    tot = disp + LAMBDA_ENTROPY * ent + LAMBDA_REPULSION * rep + LAMBDA_INTER * inter
    return tuple(np.float32(v) for v in (tot, disp, ent, rep, inter))


def _build_nc():
    import concourse.bacc as bacc
    import concourse.tile as tile
    from concourse import mybir

    f32 = mybir.dt.float32
    f16 = mybir.dt.float16
    f8 = mybir.dt.float8e4
    DR = mybir.MatmulPerfMode.DoubleRow
    AF = mybir.ActivationFunctionType
    OP = mybir.AluOpType

    nc = bacc.Bacc("TRN2", target_bir_lowering=False, debug=False,
                   enable_asserts=False, enable_partition_id=False)
    g_dram = nc.dram_tensor("g", (2 * NBLK * 128, PPB * 2 * 128), f8,
                            kind="ExternalInput").ap()
    y_dram = nc.dram_tensor("y", (128, NPAIR * 2 * W), f8, kind="ExternalInput").ap()
    out_dram = nc.dram_tensor("out", (128, 8), f32, kind="ExternalOutput").ap()
    mass_dram = nc.dram_tensor("mass", (128, 2), f32, kind="ExternalOutput").ap()

    with tile.TileContext(nc) as tc:
        with (
            tc.tile_pool(name="singles", bufs=1) as singles,
            tc.tile_pool(name="gpool", bufs=8) as gpool,
            tc.tile_pool(name="scr", bufs=2) as scr,
            tc.tile_pool(name="ph2", bufs=1) as ph2,
            tc.tile_pool(name="psacc", bufs=1, space="PSUM") as psacc,
            tc.tile_pool(name="pstmp", bufs=1, space="PSUM") as pstmp,
        ):
            yres = singles.tile([128, NPAIR * 2 * W], f8, name="yres")

            # ---- input DMA: y on the scalar HWDGE queue, g on the sync
            # HWDGE queue (~380 GB/s aggregate).  16 input transfers; the
            # 8 HWDGE sem lanes recycle input->input only (every lane
            # predecessor is a pure input transfer that completes in FIFO
            # order), so no issue ever waits on late compute.  Graduated
            # first chunks (4/12/16 pairs) let the first matmuls start
            # ~2us earlier; slab1's last two blocks are split in half so
            # the final data-gated burst is only 8 pairs.
            gtiles = [gpool.tile([128, PPB * 2 * 128], f8, name="g")
                      for _ in range(2 * NBLK)]

            def dma_g(blk, plo, phi):
                nc.sync.dma_start(
                    out=gtiles[blk][:, plo * 256:phi * 256],
                    in_=g_dram[blk * 128:(blk + 1) * 128, plo * 256:phi * 256])

            def dma_y(plo, phi):
                nc.scalar.dma_start(out=yres[:, plo * 2 * W:phi * 2 * W],
                                    in_=y_dram[:, plo * 2 * W:phi * 2 * W])

            def dma_g_sc(blk, plo, phi):
                nc.scalar.dma_start(
                    out=gtiles[blk][:, plo * 256:phi * 256],
                    in_=g_dram[blk * 128:(blk + 1) * 128, plo * 256:phi * 256])

            dma_g(0, 0, 4)
            dma_y(0, 4)
            dma_g(0, 4, 16)
            dma_y(4, 16)
            dma_g(1, 0, 16)
            dma_y(16, 32)
            dma_g(2, 0, 16)
            dma_y(32, 48)
            dma_g(3, 0, 16)
            dma_y(48, 56)
            dma_y(56, 64)
            # slab1 8-pair sub-blocks alternate sync/scalar so both queues
            # stay busy to the end and arrivals are monotone in MM order
            for blk in range(NBLK, 2 * NBLK):
                dma_g(blk, 0, 8)
                dma_g_sc(blk, 8, 16)

            # ---- PE junk source (wide: 512 moving cols ~ 213ns warm) ----
            wsrc = singles.tile([128, 512], f16)
            nc.vector.memset(wsrc, 0.0)
            wps = pstmp.tile([128, 512], f32, tag="psT0", name="warmps")

            def emit_junk(n):
                for _ in range(n):
                    nc.tensor.matmul(wps, wsrc[:, 0:128], wsrc,
                                     start=True, stop=True)


            # ---- constants ----
            ones128 = singles.tile([128, 1], f32)
            nc.gpsimd.memset(ones128, 1.0)
            eps128 = singles.tile([128, 1], f32)
            nc.gpsimd.memset(eps128, EPS)
            ones16c = singles.tile([128, 1], f16)
            nc.gpsimd.memset(ones16c, 1.0)
            ones_row = singles.tile([1, 128], f16)
            nc.gpsimd.memset(ones_row, 1.0)
            mhalf16 = singles.tile([128, 1], f16)
            nc.gpsimd.memset(mhalf16, -0.5)
            qneg_sb = singles.tile([1, 2 * 128], f16)
            id16 = singles.tile([128, 128], f16)
            nc.gpsimd.memset(id16, 0.0)
            nc.gpsimd.affine_select(
                out=id16, in_=id16,
                compare_op=OP.not_equal,
                fill=1.0, base=0, pattern=[[-1, 128]], channel_multiplier=1,
            )
            # CSC-scaled identity: transpose matmuls then yield CSC*cent^T
            # directly, so centering is a single fused subtract.
            id16csc = singles.tile([128, 128], f16)
            nc.gpsimd.memset(id16csc, 0.0)
            nc.gpsimd.affine_select(
                out=id16csc, in_=id16csc,
                compare_op=OP.not_equal,
                fill=CSC, base=0, pattern=[[-1, 128]], channel_multiplier=1,
            )
            # per-half feature indicator [4, 128] and cross-feature bias
            # -B*(1-ind); identical for both halves (local structure).
            ind16 = singles.tile([4, 128], f16)
            nc.gpsimd.memset(ind16, 0.0)
            i3 = ind16.rearrange("p (blk c) -> p blk c", c=32)
            nc.gpsimd.affine_select(
                out=i3, in_=i3, compare_op=OP.not_equal,
                fill=1.0, base=0, pattern=[[1, 4], [0, 32]],
                channel_multiplier=-1,
            )
            indB = singles.tile([4, 128], f16)
            nc.gpsimd.memset(indB, -BBIAS)
            b3 = indB.rearrange("p (blk c) -> p blk c", c=32)
            nc.gpsimd.affine_select(
                out=b3, in_=b3, compare_op=OP.not_equal,
                fill=0.0, base=0, pattern=[[1, 4], [0, 32]],
                channel_multiplier=-1,
            )
            # st cols: [wv0, wv1, eall0, eall1, rep0, rep1, rx0, rx1]
            st = ph2.tile([128, 8], f32)
            nc.gpsimd.memset(st, 0.0)

            # ---- preload the Exp table once; nothing else uses a table ----
            warm = ph2.tile([1, 2], f32)
            nc.scalar.activation(out=warm[0:1, 0:1], in_=ones128[0:1, 0:1], func=AF.Exp)

            # ---- phase 1: slab-ordered DoubleRow accumulation ----
            # ps[h][:, 0:128]=SWY_h, [:,128]=mass_raw, [:,129:131]=A_hi/lo
            ps = [psacc.tile([128, W], f32, name=f"acc{h}") for h in range(2)]

            # ---- tail tiles ----
            mass = ph2.tile([128, 2], f32)
            inv = ph2.tile([128, 2], f32)
            a_ = ph2.tile([128, 2], f32)
            csq = ph2.tile([128, 2], f32)
            t1 = ph2.tile([128, 2], f32)
            cent16 = ph2.tile([128, 2 * 128], f16)
            ccT = ph2.tile([128, 2 * 128], f16)
            sqc = scr.tile([128, 2 * 128], f16, tag="sqc")
            sqf = [scr.tile([128, 128], f16, tag=f"sqf{h}", name=f"sqf{h}")
                   for h in range(2)]
            dd = [scr.tile([128, 127], f16, tag=f"dd{h}", name=f"dd{h}")
                  for h in range(2)]
            en = [ph2.tile([1, 127], f32, name=f"en{h}") for h in range(2)]

            def v_mass_inv(h):
                nc.vector.tensor_scalar_add(
                    mass[:, h:h + 1], in0=ps[h][:, 128:129], scalar1=eps128)
                nc.vector.reciprocal(inv[:, h:h + 1], mass[:, h:h + 1])

            def v_cent(h):
                hs = slice(h * 128, (h + 1) * 128)
                with nc.allow_low_precision(reason="cent fp16 for exp"):
                    nc.vector.tensor_scalar_mul(
                        cent16[:, hs], in0=ps[h][:, 0:128], scalar1=inv[:, h:h + 1])

            def v_cc_from(h, src):
                # ccT = src - src[:,0:1]  (src = CSC*cent^T via id16csc)
                hs = slice(h * 128, (h + 1) * 128)
                with nc.allow_low_precision(reason="cc fp16 for exp"):
                    nc.vector.tensor_scalar_sub(ccT[:, hs], in0=src,
                                                scalar1=src[:, 0:1])

            def v_sqf_dd(h):
                hs = slice(h * 128, (h + 1) * 128)
                with nc.allow_low_precision(reason="scaled cc^2 fits fp16"):
                    nc.vector.tensor_mul(sqf[h], ccT[:, hs], ccT[:, hs])
                    nc.vector.tensor_sub(dd[h], ccT[:, h * 128:h * 128 + 127],
                                         ccT[:, h * 128 + 1:(h + 1) * 128])
                    nc.vector.tensor_mul(dd[h], dd[h], dd[h])

            pe_q = [None, None]
            pe_e = [None, None]
            pe_nd = [None, None]

            def p_qe(h):
                hs = slice(h * 128, (h + 1) * 128)
                pe_q[h] = pstmp.tile([1, 128], f32, tag="psq", name=f"psq{h}")
                nc.tensor.matmul(pe_q[h], mhalf16, sqf[h], start=True, stop=True)
                pe_e[h] = pstmp.tile([128, 128], f32, tag=f"psE{h}", name=f"psE{h}")
                nc.tensor.matmul(pe_e[h], ccT[:, hs], ccT[:, hs],
                                 start=True, stop=False)
                nc.tensor.matmul(pe_e[h], ind16, indB, start=False, stop=False)
                nd_t = pstmp.tile([1, 128], f32, tag="psq", name=f"psnd{h}")
                pe_nd[h] = nd_t[0:1, 0:127]
                nc.tensor.matmul(pe_nd[h], ones16c, dd[h], start=True, stop=True)

            def v_qneg(h):
                with nc.allow_low_precision(reason="q fp16 rank-1 operand"):
                    nc.vector.tensor_copy(qneg_sb[0:1, h * 128:(h + 1) * 128],
                                          pe_q[h])

            def s_qneg(h):
                with nc.allow_low_precision(reason="q fp16 rank-1 operand"):
                    nc.scalar.activation(
                        out=qneg_sb[0:1, h * 128:(h + 1) * 128],
                        in_=pe_q[h], func=AF.Copy)

            def p_rank1(h):
                qn = qneg_sb[0:1, h * 128:(h + 1) * 128]
                nc.tensor.matmul(pe_e[h], ones_row, qn, start=False, stop=False)
                nc.tensor.matmul(pe_e[h], qn, ones_row, start=False, stop=True)

            def s_exps(h):
                nc.scalar.activation(out=en[h], in_=pe_nd[h], func=AF.Exp,
                                     scale=-1.0 / (CSC * CSC),
                                     accum_out=st[0:1, 4 + h:5 + h])
                e_full = scr.tile([128, 128], f16, tag=f"ef{h}", name=f"ef{h}")
                with nc.allow_low_precision(reason="E<=1 fp16; accum f32"):
                    nc.scalar.activation(out=e_full, in_=pe_e[h], func=AF.Exp,
                                         scale=2.0 / (CSC * CSC),
                                         accum_out=st[:, 2 + h:3 + h])

            def v_stats(h):
                hs = slice(h * 128, (h + 1) * 128)
                xview = en[h][0:1, 31:31 + 96].rearrange("p (m c) -> p m c", c=32)
                nc.vector.reduce_sum(st[0:1, 6 + h:7 + h], xview[:, :, 0:1],
                                     axis=mybir.AxisListType.XY)
                nc.vector.reduce_sum(
                    a_[:, h:h + 1],
                    ps[h][:, 129:131].rearrange("p (one c) -> p one c", one=1),
                    axis=mybir.AxisListType.X)
                with nc.allow_low_precision(reason="csq via fp16 cent"):
                    nc.vector.tensor_mul(sqc[:, hs], cent16[:, hs], cent16[:, hs])
                nc.vector.reduce_sum(
                    csq[:, h:h + 1],
                    sqc[:, hs].rearrange("p (one c) -> p one c", one=1),
                    axis=mybir.AxisListType.X)
                nc.vector.tensor_mul(t1[:, h:h + 1], a_[:, h:h + 1],
                                     inv[:, h:h + 1])
                nc.vector.tensor_sub(st[:, h:h + 1], t1[:, h:h + 1],
                                     csq[:, h:h + 1])

            def emit_mm(u, h, start=False, stop=False):
                blk, ul = divmod(u, PPB)
                g = gtiles[h * NBLK + blk]
                gv = g[:, ul * 256:(ul + 1) * 256].rearrange(
                    "p (i fk) -> p i fk", i=2)
                yv = yres[:, u * 2 * W:(u + 1) * 2 * W].rearrange(
                    "p (i w) -> p i w", i=2)
                nc.tensor.matmul(
                    ps[h], gv, yv, start=start, stop=stop, perf_mode=DR,
                )

            def mm_range(lo, hi, h, start=False, stop=False):
                for u in range(lo, hi):
                    emit_mm(u, h, start=(start and u == lo),
                            stop=(stop and u == hi - 1))

            # slab1-era junk uses the psE1 bank (free until pe1 ~26us) so
            # it can never stall behind h0-tail reads of psT0.
            wps2 = pstmp.tile([128, 512], f32, tag="psE1", name="warmps2")

            def emit_junk2(n):
                for _ in range(n):
                    nc.tensor.matmul(wps2, wsrc[:, 0:128], wsrc,
                                     start=True, stop=True)

            # ---- slab0 (junk fills DMA pacing gaps) ----
            # no gap junk: after the upfront ramp unlocks the clock, the
            # natural ~55-70% duty through slab0 is active enough to avoid
            # the idle re-throttle but below the sustained-load duty cap
            # (continuous 100% activity triggers K=4 after one ~6.8us epoch)
            mm_range(0, 4, 0, start=True)       # b0a
            mm_range(4, 16, 0)                  # b0b
            mm_range(16, 32, 0)                 # b1
            mm_range(32, 48, 0)                 # b2
            mm_range(48, 64, 0, stop=True)      # b3 (y-gated trickle)

            # ---- h0 tail prefix: vector chain + one PE transpose ----
            v_mass_inv(0)
            v_cent(0)
            ps_t0 = pstmp.tile([128, 128], f32, tag="psT0", name="psT0")
            nc.tensor.matmul(ps_t0, cent16[:, 0:128], id16csc, start=True, stop=True)
            v_cc_from(0, ps_t0)
            v_sqf_dd(0)

            # ---- slab1 stream with h0's remaining tail ops slotted in ----
            mm_range(0, 32, 1, start=True)
            p_qe(0)                             # PE: psq0, pe0 dots+bias, nd0
            v_qneg(0)                           # vector (after psq0)
            mm_range(32, 48, 1)
            p_rank1(0)                          # PE (after qneg0): pe0 stop
            s_exps(0)                           # scalar: en0, ef0 (+accums)
            mm_range(48, 56, 1)
            v_stats(0)                          # vector (off-critical)
            mm_range(56, 64, 1, stop=True)

            # ---- endgame: h1 half-tail only ----
            v_mass_inv(1)                       # vector
            nc.sync.dma_start(out=mass_dram, in_=mass)
            v_cent(1)                           # vector
            ps_t1 = pstmp.tile([128, 128], f32, tag="psT1", name="psT1")
            nc.tensor.matmul(ps_t1, cent16[:, 128:256], id16csc, start=True, stop=True)
            v_cc_from(1, ps_t1)                 # vector: nshift1, ccT1
            v_sqf_dd(1)                         # vector
            p_qe(1)                             # PE: psq1, pe1 dots+bias, nd1
            s_qneg(1)                           # scalar (vector busy with dd1)
            p_rank1(1)                          # PE: pe1 stop
            s_exps(1)                           # scalar: en1, ef1 (+accums)
            v_stats(1)                          # vector

            nc.sync.dma_start(out=out_dram, in_=st)

    nc.compile()
    return nc


def get_nc():
    if "v30" not in _NC_CACHE:
        _NC_CACHE["v30"] = _build_nc()
    return _NC_CACHE["v30"]


def kernel(membership: np.ndarray, teacher_preds: np.ndarray, _trace: bool = False):
    from concourse.bass_utils import run_bass_kernel_spmd

    f8 = _f8dtype()
    m = np.asarray(membership, dtype=np.float32).reshape(N, F * K)
    y32 = np.asarray(teacher_preds, dtype=np.float32)
    ysq = np.einsum("nc,nc->n", y32, y32, dtype=np.float64).astype(np.float32)
    ysq_h = ysq.astype(f8)
    ysq_l = (ysq - ysq_h.astype(np.float32)).astype(f8)
    yslab = np.zeros((N, W), dtype=f8)
    yslab[:, 0:C] = y32.astype(f8)
    yslab[:, C] = np.float32(1.0)
    yslab[:, C + 1] = ysq_h
    yslab[:, C + 2] = ysq_l
    ypacked = _pack_y(yslab)

    m8 = m.astype(f8)
    nc = get_nc()
    in_maps = []
    for i in range(NCORES):
        in_maps.append({
            "g": _pack_g(m8[:, i * FK:(i + 1) * FK]),
            "y": ypacked,
        })
    res = run_bass_kernel_spmd(
        nc, in_maps, core_ids=list(range(NCORES)), trace=_trace,
    )
    parts = np.stack(
        [np.asarray(res.results[i]["out"], dtype=np.float64) for i in range(NCORES)]
    )
    masses = np.stack(
        [np.asarray(res.results[i]["mass"], dtype=np.float64) for i in range(NCORES)]
    )
    out = _finalize(parts, masses)
    if _trace:
        return out, res
    return out


if __name__ == "__main__":
    rng = np.random.default_rng(0)
    mem = rng.random((N, F, K), dtype=np.float32)
    tp = rng.random((N, C), dtype=np.float32)
    print(kernel(mem, tp))
